# revision 18
# baseline (speedup 1.0000x reference)
"""Trainium2 Bass kernel for BERTLegalCA (nn_BERTLegalCA_90615220011635).

Math (reference):
  laws_q = laws @ Wq + bq                     [L,R,D]
  k = output @ Wk + bk; v = output @ Wv + bv  [B,S,D]
  scores = einsum('lrd,bsd->lbrs', laws_q, k)/sqrt(D) + maskbias
  probs = softmax(scores, -1)
  g = einsum('lbrs,bsd->lbrd', probs, v)
  law  = sum_r tanh(g.w_law_d  + b_law_d) * w_law1 + b_law1          [B,L]
  accu = tanh(sum_r tanh(g.w_accu_d + b_accu_d)*w_accu1 + b_accu1) @ W_accu2.T + b_accu2
  term = analogous with term weights                                  [B,11]

Key transformation: g only appears through dot products with the three
head vectors w_*_d. With vh = v @ [w_law_d|w_accu_d|w_term_d]  [B,S,3]:
the per-(l,b) work reduces to
  u[j,r]  = sum_s exp_scores^T[s,r] * vh[s,j]   (j=0..2)
  sums[r] = sum_s exp_scores^T[s,r]
one matmul with lhsT = [vh | 1,1,1] (6 cols), then
  head_j[r] = tanh(u[j,r]/sums[r] + (bv.w_j + b_j_d))
  out_j     = sum_r head_j[r] * w_j1[r]
The [L,B,R,D] tensor 'g' (83 of the 231 GFLOP) is never materialized.
Softmax max-subtraction is skipped (scores are O(1), no overflow risk);
the attention-mask bias folds into the per-partition bias of the fused
exp/PSUM-evacuation ACT op (s sits on partitions in every layout here).

Sharding: L=103 laws -> 8 cores x 13 (last core gets one zero-padded law).
Each core computes out_j for its laws; host gathers [B,103] logits and
applies the tiny accu2/term2 mixing layers.

All matmuls run as float32r (full-rate fp32 on the PE array).
The per-law emission order is: qT matmuls, scores(b0), scores(b1),
usums(b0), usums(b1) - so the exp of batch b runs on ACT behind the
other batch's scores matmuls and the PE never stalls on softmax.
"""

import math
import sys
from contextlib import ExitStack

import numpy as np

sys.path.insert(0, "/opt/trn_rl_repo")

L, R, B, S, D = 103, 512, 2, 512, 768
NCORES = 8
LC = 13          # laws per core; NCORES*LC = 104 >= L
KT = D // 128    # 6  d-tiles
SMT = S // 128   # 4  s-tiles
SQRT_D = math.sqrt(D)

_CACHE = {}
LAST_RESULT = None  # BassKernelResults of the most recent run (for test harness)
TRACE = False       # set True (by test harness) to collect an NTFF profile


def _build_nc(n_laws=LC, head_chain=True):
    import concourse.tile as tile
    from concourse import bacc, mybir

    f32 = mybir.dt.float32
    f32r = mybir.dt.float32r
    Act = mybir.ActivationFunctionType

    nc = bacc.Bacc(None, target_bir_lowering=False)

    d_laws = nc.dram_tensor("lawsT", [LC, D, R], f32, kind="ExternalInput")
    d_oT = nc.dram_tensor("oT", [B, D, S], f32, kind="ExternalInput")
    # wq/wk pre-tiled host-side as [128, m, kt, 128] so each m-slab is one
    # contiguous DMA and the first matmul only waits for one slab.
    d_wq = nc.dram_tensor("wq", [128, KT, KT, 128], f32, kind="ExternalInput")
    d_wk = nc.dram_tensor("wk", [128, KT, KT, 128], f32, kind="ExternalInput")
    d_wvh = nc.dram_tensor("wvh", [128, KT, 4], f32, kind="ExternalInput")
    d_bq = nc.dram_tensor("bq", [128, KT], f32, kind="ExternalInput")
    d_bk = nc.dram_tensor("bk", [128, KT], f32, kind="ExternalInput")
    d_maskb = nc.dram_tensor("maskb", [128, B * SMT], f32, kind="ExternalInput")
    d_ones = nc.dram_tensor("vones", [128, SMT, 3], f32, kind="ExternalInput")
    d_w1 = nc.dram_tensor("w1", [3, R], f32, kind="ExternalInput")
    d_bias3 = nc.dram_tensor("bias3", [3, 1], f32, kind="ExternalInput")
    d_res = nc.dram_tensor("res", [3, LC * B], f32, kind="ExternalOutput")

    with tile.TileContext(nc) as tc, ExitStack() as ctx:
        consts = ctx.enter_context(tc.tile_pool(name="consts", bufs=1))
        oT_pool = ctx.enter_context(tc.tile_pool(name="oT", bufs=2))
        laws_pool = ctx.enter_context(tc.tile_pool(name="laws", bufs=3))
        q_pool = ctx.enter_context(tc.tile_pool(name="qT", bufs=2))
        probs_pool = ctx.enter_context(tc.tile_pool(name="probs", bufs=3))
        scr_pool = ctx.enter_context(tc.tile_pool(name="scr", bufs=3))
        ps_pool = ctx.enter_context(tc.tile_pool(name="ps", bufs=8, space="PSUM"))

        t_wq = consts.tile([128, KT, KT, 128], f32r, tag="wq")
        t_wk = consts.tile([128, KT, KT, 128], f32r, tag="wk")
        t_oT = [oT_pool.tile([128, KT, S], f32r, tag="oT", name=f"oT{b}")
                for b in range(B)]
        # DMA issue costs ~0.65us of sequencer time per dma_start, so big
        # transfers stay whole and the issue load spreads across four
        # otherwise-idle engine queues. Only the kt0-critical pair is split
        # (across engines) to shorten the first matmul's wait.
        nc.sync.dma_start(t_oT[0][:, 0, 0:256],
                          d_oT[0, 0:128, 0:256].bitcast(f32r))
        nc.gpsimd.dma_start(t_oT[0][:, 0, 256:512],
                            d_oT[0, 0:128, 256:512].bitcast(f32r))
        nc.gpsimd.dma_start(t_wk[:, 0, 0:3], d_wk[:, 0, 0:3].bitcast(f32r))
        nc.sync.dma_start(t_wk[:, 0, 3:6], d_wk[:, 0, 3:6].bitcast(f32r))

        t_warm = consts.tile([128, 256], f32, tag="warm")
        nc.vector.memset(t_warm[:], 1.0)
        wps = [ps_pool.tile([128, 64], f32, tag="ps", name=f"warm{i}")
               for i in range(2)]
        for w in range(26):
            nc.tensor.matmul(wps[w % 2][:], t_warm[:, 0:128], t_warm[:, 0:64],
                             start=True, stop=True)
        for i in range(2):
            nc.vector.tensor_copy(t_warm[:, i : i + 1], wps[i][:, 0:1])

        # small consts (cheap DMAs, needed early)
        t_wvh = consts.tile([128, KT, 4], f32r, tag="wvh")
        nc.sync.dma_start(t_wvh[:], d_wvh[:].bitcast(f32r))
        t_bk = consts.tile([128, KT], f32, tag="bk")
        nc.sync.dma_start(t_bk[:], d_bk[:])
        t_bq = consts.tile([128, KT], f32, tag="bq")
        nc.sync.dma_start(t_bq[:], d_bq[:])
        t_maskb = consts.tile([128, B * SMT], f32, tag="maskb")
        nc.sync.dma_start(t_maskb[:], d_maskb[:])
        t_w1 = consts.tile([3, R], f32, tag="w1")
        nc.sync.dma_start(t_w1[:], d_w1[:])
        t_bias3 = consts.tile([3, 1], f32, tag="bias3")
        nc.sync.dma_start(t_bias3[:], d_bias3[:])
        t_staging = consts.tile([3, LC * B], f32, tag="staging")

        t_kT = [consts.tile([128, KT, S], f32r, tag=f"kT{b}", name=f"kT{b}")
                for b in range(B)]
        t_vh = [consts.tile([128, SMT, 6], f32r, tag=f"vh{b}", name=f"vh{b}")
                for b in range(B)]

        # ---- prologue: facts_k^T and vh per batch ----
        # kt-outer with 6 held PSUM accumulators: the first matmul only
        # waits for oT chunk kt0 + wk slab kt0 (~640KB of DMA).
        for b in range(B):
            t_o = t_oT[b]
            kt_ps = [ps_pool.tile([128, S], f32, tag="ps", name=f"kps{b}_{m}")
                     for m in range(KT)]
            for kt in range(KT):
                if b == 0:
                    if kt > 0:
                        nc.sync.dma_start(t_wk[:, kt], d_wk[:, kt].bitcast(f32r))
                        nc.sync.dma_start(
                            t_o[:, kt, :],
                            d_oT[0, kt * 128 : (kt + 1) * 128, :].bitcast(f32r)
                        )
                    nc.sync.dma_start(
                        t_oT[1][:, kt, :],
                        d_oT[1, kt * 128 : (kt + 1) * 128, :].bitcast(f32r)
                    )
                for m in range(KT):
                    nc.tensor.matmul(
                        kt_ps[m][:],
                        t_wk[:, kt, m, :],
                        t_o[:, kt, :],
                        start=(kt == 0),
                        stop=(kt == KT - 1),
                    )
            for m in range(KT):
                nc.scalar.activation(
                    t_kT[b][:, m, :], kt_ps[m][:], Act.Identity,
                    bias=t_bk[:, m : m + 1]
                )
            for sm in range(SMT):
                psv = ps_pool.tile([128, 4], f32, tag="ps")
                for kt in range(KT):
                    nc.tensor.matmul(
                        psv[:],
                        t_o[:, kt, sm * 128 : (sm + 1) * 128],
                        t_wvh[:, kt, :],
                        start=(kt == 0),
                        stop=(kt == KT - 1),
                    )
                nc.scalar.activation(t_vh[b][:, sm, 0:3], psv[:, 0:3], Act.Copy)
            nc.sync.dma_start(t_vh[b][:, :, 3:6], d_ones[:].bitcast(f32r))

        def load_law(li):
            t_l = laws_pool.tile([128, KT, R], f32r, tag="lawsT", name=f"law{li}")
            for kt in range(KT):
                nc.sync.dma_start(
                    t_l[:, kt, :],
                    d_laws[li, kt * 128 : (kt + 1) * 128, :].bitcast(f32r),
                )
            return t_l

        pending_law = load_law(0) if n_laws > 0 else None
        for m in range(KT):  # wq lands behind laws l0, before l1+
            nc.sync.dma_start(t_wq[:, m], d_wq[:, m].bitcast(f32r))

        # ---- steady state: per law ----
        for li in range(n_laws):
            t_l = pending_law
            t_q = q_pool.tile([128, KT, R], f32r, tag="qT")
            for m in range(KT):
                ps = ps_pool.tile([128, R], f32, tag="ps")
                for kt in range(KT):
                    nc.tensor.matmul(
                        ps[:],
                        t_wq[:, m, kt, :],
                        t_l[:, kt, :],
                        start=(kt == 0),
                        stop=(kt == KT - 1),
                    )
                nc.scalar.activation(
                    t_q[:, m, :], ps[:], Act.Identity, bias=t_bq[:, m : m + 1]
                )
            pending_law = load_law(li + 1) if li + 1 < n_laws else None
            t_ps = []
            for b in range(B):
                t_p = probs_pool.tile([128, SMT, R], f32r, tag="probs")
                t_ps.append(t_p)
                for sm in range(SMT):
                    ps = ps_pool.tile([128, R], f32, tag="ps")
                    for kt in range(KT):
                        nc.tensor.matmul(
                            ps[:],
                            t_kT[b][:, kt, sm * 128 : (sm + 1) * 128],
                            t_q[:, kt, :],
                            start=(kt == 0),
                            stop=(kt == KT - 1),
                        )
                    nc.scalar.activation(
                        t_p[:, sm, :],
                        ps[:],
                        Act.Exp,
                        bias=t_maskb[:, b * SMT + sm : b * SMT + sm + 1],
                    )
            uss = []
            for b in range(B):
                us = ps_pool.tile([6, R], f32, tag="ps")
                uss.append(us)
                for sm in range(SMT):
                    nc.tensor.matmul(
                        us[:],
                        t_vh[b][:, sm, :],
                        t_ps[b][:, sm, :],
                        start=(sm == 0),
                        stop=(sm == SMT - 1),
                    )
            if not head_chain:
                continue
            for b in range(B):
                us = uss[b]
                col = li * B + b
                t_us = scr_pool.tile([6, R], f32, tag="us")
                nc.scalar.activation(t_us[:], us[:], Act.Copy)
                t_sums = scr_pool.tile([3, R], f32, tag="sums")
                nc.sync.dma_start(t_sums[:], t_us[3:6, :])
                t_rec = scr_pool.tile([3, R], f32, tag="rec")
                nc.vector.reciprocal_approx_fast(out=t_rec[:], in_=t_sums[:])
                t_hd = scr_pool.tile([3, R], f32, tag="hd")
                nc.vector.tensor_mul(t_hd[:], t_us[0:3, :], t_rec[:])
                t_th = scr_pool.tile([3, R], f32, tag="th")
                nc.scalar.activation(t_th[:], t_hd[:], Act.Tanh, bias=t_bias3[:, 0:1])
                t_tt = scr_pool.tile([3, R], f32, tag="tt")
                nc.vector.tensor_mul(t_tt[:], t_th[:], t_w1[:])
                nc.vector.reduce_sum(
                    out=t_staging[:, col : col + 1], in_=t_tt[:],
                    axis=mybir.AxisListType.X,
                )
        if n_laws > 0 and head_chain:
            nc.sync.dma_start(d_res[:], t_staging[:])

    nc.compile()
    return nc


def _get_nc():
    if "nc" not in _CACHE:
        _CACHE["nc"] = _build_nc()
    return _CACHE["nc"]


def kernel(**inputs):
    global LAST_RESULT
    from concourse.bass_utils import run_bass_kernel_spmd

    f = lambda name: np.ascontiguousarray(np.asarray(inputs[name]), dtype=np.float32)
    output = f("output")            # [B,S,D]
    mask = f("attention_mask")      # [B,S]
    laws = f("laws")                # [L,R,D]
    Wq, bq = f("Wq"), f("bq")
    Wk, bk = f("Wk"), f("bk")
    Wv, bv = f("Wv"), f("bv")
    wd_stack = np.stack([f("w_law_d"), f("w_accu_d"), f("w_term_d")], axis=1)  # [D,3]
    w1_stack = np.stack([f("w_law1"), f("w_accu1"), f("w_term1")], axis=0)     # [3,R]
    bd_stack = np.array(
        [f("b_law_d")[0], f("b_accu_d")[0], f("b_term_d")[0]], np.float32
    )

    # host-side weight preprocessing
    oT = np.ascontiguousarray(output.transpose(0, 2, 1))                # [B,D,S]
    lawsT_pad = np.zeros((NCORES * LC, D, R), np.float32)
    lawsT_pad[:L] = laws.transpose(0, 2, 1)
    # [din,dout] -> [p, m, kt, j] with din = kt*128+p, dout = m*128+j
    # wq: [p, m, kt, j]; wk: [p, kt, m, j]  (din = kt*128+p, dout = m*128+j)
    wq_p = np.ascontiguousarray(
        (Wq / SQRT_D).reshape(KT, 128, KT, 128).transpose(1, 2, 0, 3))
    wk_p = np.ascontiguousarray(Wk.reshape(KT, 128, KT, 128).transpose(1, 0, 2, 3))
    wvh_p = np.ascontiguousarray(
        np.concatenate([Wv @ wd_stack, np.zeros((D, 1), np.float32)], 1)
        .reshape(KT, 128, 4).transpose(1, 0, 2))                        # [128,KT,4]
    bq_p = np.ascontiguousarray((bq / SQRT_D).reshape(KT, 128).T)       # [128,KT]
    bk_p = np.ascontiguousarray(bk.reshape(KT, 128).T)
    maskb = np.ascontiguousarray(
        ((1.0 - mask) * -10000.0).reshape(B, SMT, 128).transpose(2, 0, 1)
        .reshape(128, B * SMT))
    bias3 = (bv @ wd_stack + bd_stack).reshape(3, 1).astype(np.float32)
    vones = np.ones((128, SMT, 3), np.float32)

    common = {
        "oT": oT, "wq": wq_p, "wk": wk_p, "wvh": wvh_p, "bq": bq_p, "bk": bk_p,
        "maskb": maskb, "vones": vones, "w1": np.ascontiguousarray(w1_stack),
        "bias3": bias3,
    }
    in_maps = [
        {**common, "lawsT": np.ascontiguousarray(lawsT_pad[c * LC : (c + 1) * LC])}
        for c in range(NCORES)
    ]

    nc = _get_nc()
    kw = {"trace": True, "trace_cores": [0]} if TRACE else {}
    LAST_RESULT = run_bass_kernel_spmd(nc, in_maps, core_ids=list(range(NCORES)), **kw)

    # gather: res[c] is [3, LC*B] with col = li*B + b
    allres = np.stack([LAST_RESULT.results[c]["res"] for c in range(NCORES)])
    X = allres.reshape(NCORES, 3, LC, B).transpose(1, 3, 0, 2).reshape(3, B, NCORES * LC)
    X = X[:, :, :L]  # [3, B, L]

    law = X[0] + f("b_law1")[0]
    accu = np.tanh(X[1] + f("b_accu1")[0]) @ f("W_accu2").T + f("b_accu2")
    term = np.tanh(X[2] + f("b_term1")[0]) @ f("W_term2").T + f("b_term2")
    return law.astype(np.float32), accu.astype(np.float32), term.astype(np.float32)


# revision 19
# speedup vs baseline: 1.0024x; 1.0024x over previous
"""Trainium2 Bass kernel for BERTLegalCA (nn_BERTLegalCA_90615220011635).

Math (reference):
  laws_q = laws @ Wq + bq                     [L,R,D]
  k = output @ Wk + bk; v = output @ Wv + bv  [B,S,D]
  scores = einsum('lrd,bsd->lbrs', laws_q, k)/sqrt(D) + maskbias
  probs = softmax(scores, -1)
  g = einsum('lbrs,bsd->lbrd', probs, v)
  law  = sum_r tanh(g.w_law_d  + b_law_d) * w_law1 + b_law1          [B,L]
  accu = tanh(sum_r tanh(g.w_accu_d + b_accu_d)*w_accu1 + b_accu1) @ W_accu2.T + b_accu2
  term = analogous with term weights                                  [B,11]

Key transformation: g only appears through dot products with the three
head vectors w_*_d. With vh = v @ [w_law_d|w_accu_d|w_term_d]  [B,S,3]:
the per-(l,b) work reduces to
  u[j,r]  = sum_s exp_scores^T[s,r] * vh[s,j]   (j=0..2)
  sums[r] = sum_s exp_scores^T[s,r]
one matmul with lhsT = [vh | 1,1,1] (6 cols), then
  head_j[r] = tanh(u[j,r]/sums[r] + (bv.w_j + b_j_d))
  out_j     = sum_r head_j[r] * w_j1[r]
The [L,B,R,D] tensor 'g' (83 of the 231 GFLOP) is never materialized.
Softmax max-subtraction is skipped (scores are O(1), no overflow risk);
the attention-mask bias folds into the per-partition bias of the fused
exp/PSUM-evacuation ACT op (s sits on partitions in every layout here).

Sharding: L=103 laws -> 8 cores x 13 (last core gets one zero-padded law).
Each core computes out_j for its laws; host gathers [B,103] logits and
applies the tiny accu2/term2 mixing layers.

All matmuls run as float32r (full-rate fp32 on the PE array).
The per-law emission order is: qT matmuls, scores(b0), scores(b1),
usums(b0), usums(b1) - so the exp of batch b runs on ACT behind the
other batch's scores matmuls and the PE never stalls on softmax.
"""

import math
import sys
from contextlib import ExitStack

import numpy as np

sys.path.insert(0, "/opt/trn_rl_repo")

L, R, B, S, D = 103, 512, 2, 512, 768
NCORES = 8
LC = 13          # laws per core; NCORES*LC = 104 >= L
KT = D // 128    # 6  d-tiles
SMT = S // 128   # 4  s-tiles
SQRT_D = math.sqrt(D)

_CACHE = {}
LAST_RESULT = None  # BassKernelResults of the most recent run (for test harness)
TRACE = False       # set True (by test harness) to collect an NTFF profile


def _build_nc(n_laws=LC, head_chain=True):
    import concourse.tile as tile
    from concourse import bacc, mybir

    f32 = mybir.dt.float32
    f32r = mybir.dt.float32r
    Act = mybir.ActivationFunctionType

    nc = bacc.Bacc(None, target_bir_lowering=False)

    d_laws = nc.dram_tensor("lawsT", [LC, D, R], f32, kind="ExternalInput")
    d_oT = nc.dram_tensor("oT", [B, D, S], f32, kind="ExternalInput")
    # wq/wk pre-tiled host-side as [128, m, kt, 128] so each m-slab is one
    # contiguous DMA and the first matmul only waits for one slab.
    d_wq = nc.dram_tensor("wq", [128, KT, KT, 128], f32, kind="ExternalInput")
    d_wk = nc.dram_tensor("wk", [128, KT, KT, 128], f32, kind="ExternalInput")
    d_wvh = nc.dram_tensor("wvh", [128, KT, 4], f32, kind="ExternalInput")
    d_bq = nc.dram_tensor("bq", [128, KT], f32, kind="ExternalInput")
    d_bk = nc.dram_tensor("bk", [128, KT], f32, kind="ExternalInput")
    d_maskb = nc.dram_tensor("maskb", [128, B * SMT], f32, kind="ExternalInput")
    d_ones = nc.dram_tensor("vones", [128, SMT, 3], f32, kind="ExternalInput")
    d_w1 = nc.dram_tensor("w1", [3, R], f32, kind="ExternalInput")
    d_bias3 = nc.dram_tensor("bias3", [3, 1], f32, kind="ExternalInput")
    d_res = nc.dram_tensor("res", [3, LC * B], f32, kind="ExternalOutput")

    with tile.TileContext(nc) as tc, ExitStack() as ctx:
        consts = ctx.enter_context(tc.tile_pool(name="consts", bufs=1))
        oT_pool = ctx.enter_context(tc.tile_pool(name="oT", bufs=2))
        laws_pool = ctx.enter_context(tc.tile_pool(name="laws", bufs=3))
        q_pool = ctx.enter_context(tc.tile_pool(name="qT", bufs=2))
        probs_pool = ctx.enter_context(tc.tile_pool(name="probs", bufs=3))
        scr_pool = ctx.enter_context(tc.tile_pool(name="scr", bufs=3))
        ps_pool = ctx.enter_context(tc.tile_pool(name="ps", bufs=8, space="PSUM"))

        t_wq = consts.tile([128, KT, KT, 128], f32r, tag="wq")
        t_wk = consts.tile([128, KT, KT, 128], f32r, tag="wk")
        t_oT = [oT_pool.tile([128, KT, S], f32r, tag="oT", name=f"oT{b}")
                for b in range(B)]
        # DMA issue costs ~0.65us of sequencer time per dma_start, so big
        # transfers stay whole and the issue load spreads across four
        # otherwise-idle engine queues. Only the kt0-critical pair is split
        # (across engines) to shorten the first matmul's wait.
        nc.sync.dma_start(t_oT[0][:, 0, 0:256],
                          d_oT[0, 0:128, 0:256].bitcast(f32r))
        nc.gpsimd.dma_start(t_oT[0][:, 0, 256:512],
                            d_oT[0, 0:128, 256:512].bitcast(f32r))
        nc.gpsimd.dma_start(t_wk[:, 0, 0:3], d_wk[:, 0, 0:3].bitcast(f32r))
        nc.sync.dma_start(t_wk[:, 0, 3:6], d_wk[:, 0, 3:6].bitcast(f32r))

        # small consts (cheap DMAs, needed early)
        t_wvh = consts.tile([128, KT, 4], f32r, tag="wvh")
        nc.sync.dma_start(t_wvh[:], d_wvh[:].bitcast(f32r))
        t_bk = consts.tile([128, KT], f32, tag="bk")
        nc.sync.dma_start(t_bk[:], d_bk[:])
        t_bq = consts.tile([128, KT], f32, tag="bq")
        nc.sync.dma_start(t_bq[:], d_bq[:])
        t_maskb = consts.tile([128, B * SMT], f32, tag="maskb")
        nc.sync.dma_start(t_maskb[:], d_maskb[:])
        t_w1 = consts.tile([3, R], f32, tag="w1")
        nc.sync.dma_start(t_w1[:], d_w1[:])
        t_bias3 = consts.tile([3, 1], f32, tag="bias3")
        nc.sync.dma_start(t_bias3[:], d_bias3[:])
        t_staging = consts.tile([3, LC * B], f32, tag="staging")

        t_kT = [consts.tile([128, KT, S], f32r, tag=f"kT{b}", name=f"kT{b}")
                for b in range(B)]
        t_vh = [consts.tile([128, SMT, 6], f32r, tag=f"vh{b}", name=f"vh{b}")
                for b in range(B)]

        # ---- prologue: facts_k^T and vh per batch ----
        # kt-outer with 6 held PSUM accumulators: the first matmul only
        # waits for oT chunk kt0 + wk slab kt0 (~640KB of DMA).
        for b in range(B):
            t_o = t_oT[b]
            kt_ps = [ps_pool.tile([128, S], f32, tag="ps", name=f"kps{b}_{m}")
                     for m in range(KT)]
            for kt in range(KT):
                if b == 0:
                    if kt > 0:
                        nc.sync.dma_start(t_wk[:, kt], d_wk[:, kt].bitcast(f32r))
                        nc.sync.dma_start(
                            t_o[:, kt, :],
                            d_oT[0, kt * 128 : (kt + 1) * 128, :].bitcast(f32r)
                        )
                    nc.sync.dma_start(
                        t_oT[1][:, kt, :],
                        d_oT[1, kt * 128 : (kt + 1) * 128, :].bitcast(f32r)
                    )
                for m in range(KT):
                    nc.tensor.matmul(
                        kt_ps[m][:],
                        t_wk[:, kt, m, :],
                        t_o[:, kt, :],
                        start=(kt == 0),
                        stop=(kt == KT - 1),
                    )
            for m in range(KT):
                nc.scalar.activation(
                    t_kT[b][:, m, :], kt_ps[m][:], Act.Identity,
                    bias=t_bk[:, m : m + 1]
                )
            for sm in range(SMT):
                psv = ps_pool.tile([128, 4], f32, tag="ps")
                for kt in range(KT):
                    nc.tensor.matmul(
                        psv[:],
                        t_o[:, kt, sm * 128 : (sm + 1) * 128],
                        t_wvh[:, kt, :],
                        start=(kt == 0),
                        stop=(kt == KT - 1),
                    )
                nc.scalar.activation(t_vh[b][:, sm, 0:3], psv[:, 0:3], Act.Copy)
            nc.sync.dma_start(t_vh[b][:, :, 3:6], d_ones[:].bitcast(f32r))

        def load_law(li):
            t_l = laws_pool.tile([128, KT, R], f32r, tag="lawsT", name=f"law{li}")
            for kt in range(KT):
                nc.sync.dma_start(
                    t_l[:, kt, :],
                    d_laws[li, kt * 128 : (kt + 1) * 128, :].bitcast(f32r),
                )
            return t_l

        pending_law = load_law(0) if n_laws > 0 else None
        for m in range(KT):  # wq lands behind laws l0, before l1+
            nc.sync.dma_start(t_wq[:, m], d_wq[:, m].bitcast(f32r))

        # ---- steady state: per law ----
        for li in range(n_laws):
            t_l = pending_law
            t_q = q_pool.tile([128, KT, R], f32r, tag="qT")
            for m in range(KT):
                ps = ps_pool.tile([128, R], f32, tag="ps")
                for kt in range(KT):
                    nc.tensor.matmul(
                        ps[:],
                        t_wq[:, m, kt, :],
                        t_l[:, kt, :],
                        start=(kt == 0),
                        stop=(kt == KT - 1),
                    )
                nc.scalar.activation(
                    t_q[:, m, :], ps[:], Act.Identity, bias=t_bq[:, m : m + 1]
                )
            pending_law = load_law(li + 1) if li + 1 < n_laws else None
            t_ps = []
            for b in range(B):
                t_p = probs_pool.tile([128, SMT, R], f32r, tag="probs")
                t_ps.append(t_p)
                for sm in range(SMT):
                    ps = ps_pool.tile([128, R], f32, tag="ps")
                    for kt in range(KT):
                        nc.tensor.matmul(
                            ps[:],
                            t_kT[b][:, kt, sm * 128 : (sm + 1) * 128],
                            t_q[:, kt, :],
                            start=(kt == 0),
                            stop=(kt == KT - 1),
                        )
                    nc.scalar.activation(
                        t_p[:, sm, :],
                        ps[:],
                        Act.Exp,
                        bias=t_maskb[:, b * SMT + sm : b * SMT + sm + 1],
                    )
            uss = []
            for b in range(B):
                us = ps_pool.tile([6, R], f32, tag="ps")
                uss.append(us)
                for sm in range(SMT):
                    nc.tensor.matmul(
                        us[:],
                        t_vh[b][:, sm, :],
                        t_ps[b][:, sm, :],
                        start=(sm == 0),
                        stop=(sm == SMT - 1),
                    )
            if not head_chain:
                continue
            for b in range(B):
                us = uss[b]
                col = li * B + b
                t_us = scr_pool.tile([6, R], f32, tag="us")
                nc.scalar.activation(t_us[:], us[:], Act.Copy)
                t_sums = scr_pool.tile([3, R], f32, tag="sums")
                nc.sync.dma_start(t_sums[:], t_us[3:6, :])
                t_rec = scr_pool.tile([3, R], f32, tag="rec")
                nc.vector.reciprocal_approx_fast(out=t_rec[:], in_=t_sums[:])
                t_hd = scr_pool.tile([3, R], f32, tag="hd")
                nc.vector.tensor_mul(t_hd[:], t_us[0:3, :], t_rec[:])
                t_th = scr_pool.tile([3, R], f32, tag="th")
                nc.scalar.activation(t_th[:], t_hd[:], Act.Tanh, bias=t_bias3[:, 0:1])
                t_tt = scr_pool.tile([3, R], f32, tag="tt")
                nc.vector.tensor_mul(t_tt[:], t_th[:], t_w1[:])
                nc.vector.reduce_sum(
                    out=t_staging[:, col : col + 1], in_=t_tt[:],
                    axis=mybir.AxisListType.X,
                )
        if n_laws > 0 and head_chain:
            nc.sync.dma_start(d_res[:], t_staging[:])

    nc.compile()
    return nc


def _get_nc():
    if "nc" not in _CACHE:
        _CACHE["nc"] = _build_nc()
    return _CACHE["nc"]


def kernel(**inputs):
    global LAST_RESULT
    from concourse.bass_utils import run_bass_kernel_spmd

    f = lambda name: np.ascontiguousarray(np.asarray(inputs[name]), dtype=np.float32)
    output = f("output")            # [B,S,D]
    mask = f("attention_mask")      # [B,S]
    laws = f("laws")                # [L,R,D]
    Wq, bq = f("Wq"), f("bq")
    Wk, bk = f("Wk"), f("bk")
    Wv, bv = f("Wv"), f("bv")
    wd_stack = np.stack([f("w_law_d"), f("w_accu_d"), f("w_term_d")], axis=1)  # [D,3]
    w1_stack = np.stack([f("w_law1"), f("w_accu1"), f("w_term1")], axis=0)     # [3,R]
    bd_stack = np.array(
        [f("b_law_d")[0], f("b_accu_d")[0], f("b_term_d")[0]], np.float32
    )

    # host-side weight preprocessing
    oT = np.ascontiguousarray(output.transpose(0, 2, 1))                # [B,D,S]
    lawsT_pad = np.zeros((NCORES * LC, D, R), np.float32)
    lawsT_pad[:L] = laws.transpose(0, 2, 1)
    # [din,dout] -> [p, m, kt, j] with din = kt*128+p, dout = m*128+j
    # wq: [p, m, kt, j]; wk: [p, kt, m, j]  (din = kt*128+p, dout = m*128+j)
    wq_p = np.ascontiguousarray(
        (Wq / SQRT_D).reshape(KT, 128, KT, 128).transpose(1, 2, 0, 3))
    wk_p = np.ascontiguousarray(Wk.reshape(KT, 128, KT, 128).transpose(1, 0, 2, 3))
    wvh_p = np.ascontiguousarray(
        np.concatenate([Wv @ wd_stack, np.zeros((D, 1), np.float32)], 1)
        .reshape(KT, 128, 4).transpose(1, 0, 2))                        # [128,KT,4]
    bq_p = np.ascontiguousarray((bq / SQRT_D).reshape(KT, 128).T)       # [128,KT]
    bk_p = np.ascontiguousarray(bk.reshape(KT, 128).T)
    maskb = np.ascontiguousarray(
        ((1.0 - mask) * -10000.0).reshape(B, SMT, 128).transpose(2, 0, 1)
        .reshape(128, B * SMT))
    bias3 = (bv @ wd_stack + bd_stack).reshape(3, 1).astype(np.float32)
    vones = np.ones((128, SMT, 3), np.float32)

    common = {
        "oT": oT, "wq": wq_p, "wk": wk_p, "wvh": wvh_p, "bq": bq_p, "bk": bk_p,
        "maskb": maskb, "vones": vones, "w1": np.ascontiguousarray(w1_stack),
        "bias3": bias3,
    }
    in_maps = [
        {**common, "lawsT": np.ascontiguousarray(lawsT_pad[c * LC : (c + 1) * LC])}
        for c in range(NCORES)
    ]

    nc = _get_nc()
    kw = {"trace": True, "trace_cores": [0]} if TRACE else {}
    LAST_RESULT = run_bass_kernel_spmd(nc, in_maps, core_ids=list(range(NCORES)), **kw)

    # gather: res[c] is [3, LC*B] with col = li*B + b
    allres = np.stack([LAST_RESULT.results[c]["res"] for c in range(NCORES)])
    X = allres.reshape(NCORES, 3, LC, B).transpose(1, 3, 0, 2).reshape(3, B, NCORES * LC)
    X = X[:, :, :L]  # [3, B, L]

    law = X[0] + f("b_law1")[0]
    accu = np.tanh(X[1] + f("b_accu1")[0]) @ f("W_accu2").T + f("b_accu2")
    term = np.tanh(X[2] + f("b_term1")[0]) @ f("W_term2").T + f("b_term2")
    return law.astype(np.float32), accu.astype(np.float32), term.astype(np.float32)


# revision 20
# speedup vs baseline: 1.0169x; 1.0145x over previous
"""Trainium2 Bass kernel for BERTLegalCA (nn_BERTLegalCA_90615220011635).

Math (reference):
  laws_q = laws @ Wq + bq                     [L,R,D]
  k = output @ Wk + bk; v = output @ Wv + bv  [B,S,D]
  scores = einsum('lrd,bsd->lbrs', laws_q, k)/sqrt(D) + maskbias
  probs = softmax(scores, -1)
  g = einsum('lbrs,bsd->lbrd', probs, v)
  law  = sum_r tanh(g.w_law_d  + b_law_d) * w_law1 + b_law1          [B,L]
  accu = tanh(sum_r tanh(g.w_accu_d + b_accu_d)*w_accu1 + b_accu1) @ W_accu2.T + b_accu2
  term = analogous with term weights                                  [B,11]

Key transformation: g only appears through dot products with the three
head vectors w_*_d. With vh = v @ [w_law_d|w_accu_d|w_term_d]  [B,S,3]:
the per-(l,b) work reduces to
  u[j,r]  = sum_s exp_scores^T[s,r] * vh[s,j]   (j=0..2)
  sums[r] = sum_s exp_scores^T[s,r]
one matmul with lhsT = [vh | 1,1,1] (6 cols), then
  head_j[r] = tanh(u[j,r]/sums[r] + (bv.w_j + b_j_d))
  out_j     = sum_r head_j[r] * w_j1[r]
The [L,B,R,D] tensor 'g' (83 of the 231 GFLOP) is never materialized.
Softmax max-subtraction is skipped (scores are O(1), no overflow risk);
the attention-mask bias folds into the per-partition bias of the fused
exp/PSUM-evacuation ACT op (s sits on partitions in every layout here).

Sharding: L=103 laws -> 8 cores x 13 (last core gets one zero-padded law).
Each core computes out_j for its laws; host gathers [B,103] logits and
applies the tiny accu2/term2 mixing layers.

All matmuls run as float32r (full-rate fp32 on the PE array).
The per-law emission order is: qT matmuls, scores(b0), scores(b1),
usums(b0), usums(b1) - so the exp of batch b runs on ACT behind the
other batch's scores matmuls and the PE never stalls on softmax.
"""

import math
import sys
from contextlib import ExitStack

import numpy as np

sys.path.insert(0, "/opt/trn_rl_repo")

L, R, B, S, D = 103, 512, 2, 512, 768
NCORES = 8
LC = 13          # laws per core; NCORES*LC = 104 >= L
KT = D // 128    # 6  d-tiles
SMT = S // 128   # 4  s-tiles
SQRT_D = math.sqrt(D)

_CACHE = {}
LAST_RESULT = None  # BassKernelResults of the most recent run (for test harness)
TRACE = False       # set True (by test harness) to collect an NTFF profile


def _build_nc(n_laws=LC, head_chain=True):
    import concourse.tile as tile
    from concourse import bacc, mybir

    f32 = mybir.dt.float32
    f32r = mybir.dt.float32r
    Act = mybir.ActivationFunctionType

    nc = bacc.Bacc(None, target_bir_lowering=False)

    d_laws = nc.dram_tensor("lawsT", [LC, D, R], f32, kind="ExternalInput")
    d_oT = nc.dram_tensor("oT", [B, D, S], f32, kind="ExternalInput")
    # wq/wk pre-tiled host-side as [128, m, kt, 128] so each m-slab is one
    # contiguous DMA and the first matmul only waits for one slab.
    d_wq = nc.dram_tensor("wq", [128, KT, KT, 128], f32, kind="ExternalInput")
    d_wk = nc.dram_tensor("wk", [128, KT, KT, 128], f32, kind="ExternalInput")
    d_wvh = nc.dram_tensor("wvh", [128, KT, 4], f32, kind="ExternalInput")
    d_bq = nc.dram_tensor("bq", [128, KT], f32, kind="ExternalInput")
    d_bk = nc.dram_tensor("bk", [128, KT], f32, kind="ExternalInput")
    d_maskb = nc.dram_tensor("maskb", [128, B * SMT], f32, kind="ExternalInput")
    d_ones = nc.dram_tensor("vones", [128, SMT, 3], f32, kind="ExternalInput")
    d_w1 = nc.dram_tensor("w1", [3, R], f32, kind="ExternalInput")
    d_bias3 = nc.dram_tensor("bias3", [3, 1], f32, kind="ExternalInput")
    d_res = nc.dram_tensor("res", [3, LC * B], f32, kind="ExternalOutput")

    with tile.TileContext(nc) as tc, ExitStack() as ctx:
        consts = ctx.enter_context(tc.tile_pool(name="consts", bufs=1))
        oT_pool = ctx.enter_context(tc.tile_pool(name="oT", bufs=2))
        laws_pool = ctx.enter_context(tc.tile_pool(name="laws", bufs=3))
        q_pool = ctx.enter_context(tc.tile_pool(name="qT", bufs=2))
        probs_pool = ctx.enter_context(tc.tile_pool(name="probs", bufs=3))
        scr_pool = ctx.enter_context(tc.tile_pool(name="scr", bufs=3))
        ps_pool = ctx.enter_context(tc.tile_pool(name="ps", bufs=8, space="PSUM"))

        t_wq = consts.tile([128, KT, KT, 128], f32r, tag="wq")
        t_wk = consts.tile([128, KT, KT, 128], f32r, tag="wk")
        t_oT = [oT_pool.tile([128, KT, S], f32r, tag="oT", name=f"oT{b}")
                for b in range(B)]
        # DMA issue costs ~0.65us of sequencer time per dma_start, so big
        # transfers stay whole and the issue load spreads across four
        # otherwise-idle engine queues. Only the kt0-critical pair is split
        # (across engines) to shorten the first matmul's wait.
        nc.sync.dma_start(t_oT[0][:, 0, 0:256],
                          d_oT[0, 0:128, 0:256].bitcast(f32r))
        nc.gpsimd.dma_start(t_oT[0][:, 0, 256:512],
                            d_oT[0, 0:128, 256:512].bitcast(f32r))
        nc.gpsimd.dma_start(t_wk[:, 0, 0:3], d_wk[:, 0, 0:3].bitcast(f32r))
        nc.sync.dma_start(t_wk[:, 0, 3:6], d_wk[:, 0, 3:6].bitcast(f32r))

        # small consts (cheap DMAs, needed early)
        t_wvh = consts.tile([128, KT, 4], f32r, tag="wvh")
        nc.sync.dma_start(t_wvh[:], d_wvh[:].bitcast(f32r))
        t_bk = consts.tile([128, KT], f32, tag="bk")
        nc.sync.dma_start(t_bk[:], d_bk[:])
        t_bq = consts.tile([128, KT], f32, tag="bq")
        nc.sync.dma_start(t_bq[:], d_bq[:])
        t_maskb = consts.tile([128, B * SMT], f32, tag="maskb")
        nc.sync.dma_start(t_maskb[:], d_maskb[:])
        t_w1 = consts.tile([3, R], f32, tag="w1")
        nc.sync.dma_start(t_w1[:], d_w1[:])
        t_bias3 = consts.tile([3, 1], f32, tag="bias3")
        nc.sync.dma_start(t_bias3[:], d_bias3[:])
        t_staging = consts.tile([3, LC * B], f32, tag="staging")

        t_kT = [consts.tile([128, KT, S], f32r, tag=f"kT{b}", name=f"kT{b}")
                for b in range(B)]
        t_vh = [consts.tile([128, SMT, 6], f32r, tag=f"vh{b}", name=f"vh{b}")
                for b in range(B)]

        # ---- prologue: facts_k^T and vh per batch ----
        # kt-outer with 6 held PSUM accumulators: the first matmul only
        # waits for oT chunk kt0 + wk slab kt0 (~640KB of DMA).
        for b in range(B):
            t_o = t_oT[b]
            kt_ps = [ps_pool.tile([128, S], f32, tag="ps", name=f"kps{b}_{m}")
                     for m in range(KT)]
            for kt in range(KT):
                if b == 0 and kt > 0:
                    nc.sync.dma_start(t_wk[:, kt], d_wk[:, kt].bitcast(f32r))
                if not (b == 0 and kt == 0):
                    nc.sync.dma_start(
                        t_o[:, kt, :],
                        d_oT[b, kt * 128 : (kt + 1) * 128, :].bitcast(f32r)
                    )
                for m in range(KT):
                    nc.tensor.matmul(
                        kt_ps[m][:],
                        t_wk[:, kt, m, :],
                        t_o[:, kt, :],
                        start=(kt == 0),
                        stop=(kt == KT - 1),
                    )
            for m in range(KT):
                nc.scalar.activation(
                    t_kT[b][:, m, :], kt_ps[m][:], Act.Identity,
                    bias=t_bk[:, m : m + 1]
                )
            for sm in range(SMT):
                psv = ps_pool.tile([128, 4], f32, tag="ps")
                for kt in range(KT):
                    nc.tensor.matmul(
                        psv[:],
                        t_o[:, kt, sm * 128 : (sm + 1) * 128],
                        t_wvh[:, kt, :],
                        start=(kt == 0),
                        stop=(kt == KT - 1),
                    )
                nc.scalar.activation(t_vh[b][:, sm, 0:3], psv[:, 0:3], Act.Copy)
            nc.sync.dma_start(t_vh[b][:, :, 3:6], d_ones[:].bitcast(f32r))

        def load_law(li):
            t_l = laws_pool.tile([128, KT, R], f32r, tag="lawsT", name=f"law{li}")
            for kt in range(KT):
                nc.sync.dma_start(
                    t_l[:, kt, :],
                    d_laws[li, kt * 128 : (kt + 1) * 128, :].bitcast(f32r),
                )
            return t_l

        pending_law = load_law(0) if n_laws > 0 else None
        for m in range(KT):  # wq lands behind laws l0, before l1+
            nc.sync.dma_start(t_wq[:, m], d_wq[:, m].bitcast(f32r))

        # ---- steady state: per law ----
        for li in range(n_laws):
            t_l = pending_law
            t_q = q_pool.tile([128, KT, R], f32r, tag="qT")
            for m in range(KT):
                ps = ps_pool.tile([128, R], f32, tag="ps")
                for kt in range(KT):
                    nc.tensor.matmul(
                        ps[:],
                        t_wq[:, m, kt, :],
                        t_l[:, kt, :],
                        start=(kt == 0),
                        stop=(kt == KT - 1),
                    )
                nc.scalar.activation(
                    t_q[:, m, :], ps[:], Act.Identity, bias=t_bq[:, m : m + 1]
                )
            pending_law = load_law(li + 1) if li + 1 < n_laws else None
            t_ps = []
            for b in range(B):
                t_p = probs_pool.tile([128, SMT, R], f32r, tag="probs")
                t_ps.append(t_p)
                for sm in range(SMT):
                    ps = ps_pool.tile([128, R], f32, tag="ps")
                    for kt in range(KT):
                        nc.tensor.matmul(
                            ps[:],
                            t_kT[b][:, kt, sm * 128 : (sm + 1) * 128],
                            t_q[:, kt, :],
                            start=(kt == 0),
                            stop=(kt == KT - 1),
                        )
                    nc.scalar.activation(
                        t_p[:, sm, :],
                        ps[:],
                        Act.Exp,
                        bias=t_maskb[:, b * SMT + sm : b * SMT + sm + 1],
                    )
            uss = []
            for b in range(B):
                us = ps_pool.tile([6, R], f32, tag="ps")
                uss.append(us)
                for sm in range(SMT):
                    nc.tensor.matmul(
                        us[:],
                        t_vh[b][:, sm, :],
                        t_ps[b][:, sm, :],
                        start=(sm == 0),
                        stop=(sm == SMT - 1),
                    )
            if not head_chain:
                continue
            for b in range(B):
                us = uss[b]
                col = li * B + b
                t_us = scr_pool.tile([6, R], f32, tag="us")
                nc.scalar.activation(t_us[:], us[:], Act.Copy)
                t_sums = scr_pool.tile([3, R], f32, tag="sums")
                nc.sync.dma_start(t_sums[:], t_us[3:6, :])
                t_rec = scr_pool.tile([3, R], f32, tag="rec")
                nc.vector.reciprocal_approx_fast(out=t_rec[:], in_=t_sums[:])
                t_hd = scr_pool.tile([3, R], f32, tag="hd")
                nc.vector.tensor_mul(t_hd[:], t_us[0:3, :], t_rec[:])
                t_th = scr_pool.tile([3, R], f32, tag="th")
                nc.scalar.activation(t_th[:], t_hd[:], Act.Tanh, bias=t_bias3[:, 0:1])
                t_tt = scr_pool.tile([3, R], f32, tag="tt")
                nc.vector.tensor_mul(t_tt[:], t_th[:], t_w1[:])
                nc.vector.reduce_sum(
                    out=t_staging[:, col : col + 1], in_=t_tt[:],
                    axis=mybir.AxisListType.X,
                )
        if n_laws > 0 and head_chain:
            nc.sync.dma_start(d_res[:], t_staging[:])

    nc.compile()
    return nc


def _get_nc():
    if "nc" not in _CACHE:
        _CACHE["nc"] = _build_nc()
    return _CACHE["nc"]


def kernel(**inputs):
    global LAST_RESULT
    from concourse.bass_utils import run_bass_kernel_spmd

    f = lambda name: np.ascontiguousarray(np.asarray(inputs[name]), dtype=np.float32)
    output = f("output")            # [B,S,D]
    mask = f("attention_mask")      # [B,S]
    laws = f("laws")                # [L,R,D]
    Wq, bq = f("Wq"), f("bq")
    Wk, bk = f("Wk"), f("bk")
    Wv, bv = f("Wv"), f("bv")
    wd_stack = np.stack([f("w_law_d"), f("w_accu_d"), f("w_term_d")], axis=1)  # [D,3]
    w1_stack = np.stack([f("w_law1"), f("w_accu1"), f("w_term1")], axis=0)     # [3,R]
    bd_stack = np.array(
        [f("b_law_d")[0], f("b_accu_d")[0], f("b_term_d")[0]], np.float32
    )

    # host-side weight preprocessing
    oT = np.ascontiguousarray(output.transpose(0, 2, 1))                # [B,D,S]
    lawsT_pad = np.zeros((NCORES * LC, D, R), np.float32)
    lawsT_pad[:L] = laws.transpose(0, 2, 1)
    # [din,dout] -> [p, m, kt, j] with din = kt*128+p, dout = m*128+j
    # wq: [p, m, kt, j]; wk: [p, kt, m, j]  (din = kt*128+p, dout = m*128+j)
    wq_p = np.ascontiguousarray(
        (Wq / SQRT_D).reshape(KT, 128, KT, 128).transpose(1, 2, 0, 3))
    wk_p = np.ascontiguousarray(Wk.reshape(KT, 128, KT, 128).transpose(1, 0, 2, 3))
    wvh_p = np.ascontiguousarray(
        np.concatenate([Wv @ wd_stack, np.zeros((D, 1), np.float32)], 1)
        .reshape(KT, 128, 4).transpose(1, 0, 2))                        # [128,KT,4]
    bq_p = np.ascontiguousarray((bq / SQRT_D).reshape(KT, 128).T)       # [128,KT]
    bk_p = np.ascontiguousarray(bk.reshape(KT, 128).T)
    maskb = np.ascontiguousarray(
        ((1.0 - mask) * -10000.0).reshape(B, SMT, 128).transpose(2, 0, 1)
        .reshape(128, B * SMT))
    bias3 = (bv @ wd_stack + bd_stack).reshape(3, 1).astype(np.float32)
    vones = np.ones((128, SMT, 3), np.float32)

    common = {
        "oT": oT, "wq": wq_p, "wk": wk_p, "wvh": wvh_p, "bq": bq_p, "bk": bk_p,
        "maskb": maskb, "vones": vones, "w1": np.ascontiguousarray(w1_stack),
        "bias3": bias3,
    }
    in_maps = [
        {**common, "lawsT": np.ascontiguousarray(lawsT_pad[c * LC : (c + 1) * LC])}
        for c in range(NCORES)
    ]

    nc = _get_nc()
    kw = {"trace": True, "trace_cores": [0]} if TRACE else {}
    LAST_RESULT = run_bass_kernel_spmd(nc, in_maps, core_ids=list(range(NCORES)), **kw)

    # gather: res[c] is [3, LC*B] with col = li*B + b
    allres = np.stack([LAST_RESULT.results[c]["res"] for c in range(NCORES)])
    X = allres.reshape(NCORES, 3, LC, B).transpose(1, 3, 0, 2).reshape(3, B, NCORES * LC)
    X = X[:, :, :L]  # [3, B, L]

    law = X[0] + f("b_law1")[0]
    accu = np.tanh(X[1] + f("b_accu1")[0]) @ f("W_accu2").T + f("b_accu2")
    term = np.tanh(X[2] + f("b_term1")[0]) @ f("W_term2").T + f("b_term2")
    return law.astype(np.float32), accu.astype(np.float32), term.astype(np.float32)
